# revision 1
# baseline (speedup 1.0000x reference)
"""MoE-LoRA linear kernel for Trainium2 (8 NeuronCores, data-parallel over tokens).

Computes, for x:[B,S,Din], base_w:[Dout,Din], gate_w:[E,Din],
lora_A:[E*R,Din], lora_B:[Dout,E*R]:

    base   = x @ base_w.T
    logits = x @ gate_w.T ; top-2 renormalized softmax -> dense w:[*,E]
    ax     = x @ lora_A.T                 (per-expert rank-R blocks)
    delta  = (ax * w_expanded) @ lora_B.T * SCALING
    out    = base + delta

Sharding: tokens (B*S=8192) split across 8 cores, 1024 tokens each.
Weights replicated. No collectives.

On-chip per core:
  phase 1a: x streamed once as fp32 [d, t-tile]; gating logits in true fp32
            (top-2 via DVE Max8, renormalized via sigmoid identity, dense
            weights via equality masks); each x tile then copied on-chip to
            the persistent fp32r x buffer feeding every other matmul.
  phase 1b: ax in fp32r (full PE speed), gate-weight multiply, PE transpose
            to [r, t] layout for the delta matmul.
  phase 2:  per 512-wide output tile: 32 base matmuls + 4 delta matmuls
            accumulate into one PSUM bank, copy out. All fp32r.

SCALING is folded into lora_B host-side.
"""
import sys

if "/opt/trn_rl_repo" not in sys.path:
    sys.path.insert(0, "/opt/trn_rl_repo")

import numpy as np

import concourse.bacc as bacc
import concourse.mybir as mybir
import concourse.tile as tile
from concourse import bass_utils
from concourse.bass import ds, ts

B, S, DIN, DOUT = 4, 2048, 4096, 4096
E, R = 32, 16
SCALING = 2.0
NCORES = 8
T = (B * S) // NCORES  # 1024 tokens per core
P = 128
TT = T // P            # 8 token tiles
KT = DIN // P          # 32 contraction tiles
OT = DOUT // 512       # 8 output column tiles
RR = (E * R) // P      # 4 rank tiles
KC = 16                # base-weight chunks per o-tile (2 k-slices each)
F32 = mybir.dt.float32
F32R = mybir.dt.float32r

_CACHE = {}


def _build():
    nc = bacc.Bacc("TRN2", target_bir_lowering=False, debug=False)
    xT = nc.dram_tensor("xT", [DIN, T], F32, kind="ExternalInput")
    bwT = nc.dram_tensor("bwT", [DIN, DOUT], F32R, kind="ExternalInput")
    gwT = nc.dram_tensor("gwT", [DIN, E], F32, kind="ExternalInput")
    laT = nc.dram_tensor("laT", [DIN, E * R], F32R, kind="ExternalInput")
    lbT = nc.dram_tensor("lbT", [E * R, DOUT], F32R, kind="ExternalInput")
    iden = nc.dram_tensor("iden", [P, P], F32R, kind="ExternalInput")
    out = nc.dram_tensor("out", [T, DOUT], F32, kind="ExternalOutput")

    xT3 = xT.ap().rearrange("(k p) t -> p k t", p=P)
    gwT3 = gwT.ap().rearrange("(k p) e -> p k e", p=P)
    laT3 = laT.ap().rearrange("(k p) r -> p k r", p=P)
    lbT3 = lbT.ap().rearrange("(rr p) o -> p rr o", p=P)
    bwT2 = bwT.ap()
    out2 = out.ap()

    with tile.TileContext(nc, pool_alloc_mode="queue") as tc:
        with (
            tc.tile_pool(name="base", bufs=1) as bp,
            tc.tile_pool(name="psum", bufs=8, space="PSUM") as psum,
        ):
            identity = bp.tile([P, P], F32R, tag="iden")
            xsb = bp.tile([P, KT, T], F32R, tag="xsb")
            axwT = bp.tile([P, RR, T], F32R, tag="axwT")
            wdense = []
            for t in range(TT):
                wd = bp.tile([P, E], F32, tag=f"wd{t}", name=f"wd{t}")
                wdense.append(wd)

            # ---- phase 1a: stream x once (fp32); gating + on-chip fp32r copy
            with tc.tile_pool(name="p1a", bufs=2) as p1a:
                KH = KT // 2
                gwt = p1a.tile([P, KT, E], F32, tag="gw", bufs=1)
                nc.sync.dma_start(gwt[:, :KH, :], gwT3[:, :KH, :])
                gw_hi_loaded = False
                for t in range(TT):
                    pl = psum.tile([P, E], F32, tag="bank", name="pl")
                    for h in range(2):
                        x32 = p1a.tile(
                            [P, KH, P], F32, tag="x32", name="x32", bufs=3
                        )
                        nc.sync.dma_start(
                            x32[:], xT3[:, ds(h * KH, KH), ts(t, P)]
                        )
                        if not gw_hi_loaded:
                            nc.sync.dma_start(
                                gwt[:, KH:, :], gwT3[:, KH:, :]
                            )
                            gw_hi_loaded = True
                        for k in range(KH):
                            nc.tensor.matmul(
                                pl[:], x32[:, k, :], gwt[:, h * KH + k, :],
                                start=(h == 0 and k == 0),
                                stop=(h == 1 and k == KH - 1),
                            )
                        # persist the fp32r copy for all later matmuls
                        # (GPSIMD: keeps DVE free so the x32 slot recycles
                        # without stalling the next tile's DMA)
                        nc.gpsimd.tensor_copy(
                            xsb[:, ds(h * KH, KH), ts(t, P)],
                            x32[:].bitcast(F32R),
                        )
                    lsb = p1a.tile([P, E], F32, tag="lsb", name="lsb")
                    nc.vector.tensor_copy(lsb[:], pl[:])
                    m8 = p1a.tile([P, 8], F32, tag="m8", name="m8")
                    nc.vector.max(out=m8[:], in_=lsb[:])
                    d21 = p1a.tile([P, 1], F32, tag="d21", name="d21")
                    nc.vector.tensor_sub(d21[:], m8[:, 1:2], m8[:, 0:1])
                    e2 = p1a.tile([P, 1], F32, tag="e2", name="e2")
                    nc.scalar.activation(
                        e2[:], d21[:], mybir.ActivationFunctionType.Exp
                    )
                    den = p1a.tile([P, 1], F32, tag="den", name="den")
                    nc.vector.tensor_scalar_add(den[:], e2[:], 1.0)
                    w1 = p1a.tile([P, 1], F32, tag="w1", name="w1")
                    nc.vector.reciprocal(w1[:], den[:])
                    w2 = p1a.tile([P, 1], F32, tag="w2", name="w2")
                    nc.vector.tensor_mul(w2[:], e2[:], w1[:])
                    eq1 = p1a.tile([P, E], F32, tag="eq1", name="eq1")
                    nc.vector.tensor_tensor(
                        eq1[:], lsb[:], m8[:, 0:1].to_broadcast([P, E]),
                        mybir.AluOpType.is_equal,
                    )
                    eq2 = p1a.tile([P, E], F32, tag="eq2", name="eq2")
                    nc.vector.tensor_tensor(
                        eq2[:], lsb[:], m8[:, 1:2].to_broadcast([P, E]),
                        mybir.AluOpType.is_equal,
                    )
                    nc.vector.tensor_tensor(
                        eq1[:], eq1[:], w1[:].to_broadcast([P, E]),
                        mybir.AluOpType.mult,
                    )
                    nc.vector.tensor_tensor(
                        eq2[:], eq2[:], w2[:].to_broadcast([P, E]),
                        mybir.AluOpType.mult,
                    )
                    nc.vector.tensor_add(wdense[t][:], eq1[:], eq2[:])

            # ---- phase 1b: ax (fp32r), gate multiply, transpose ----
            nc.sync.dma_start(identity[:], iden.ap())
            with tc.tile_pool(name="p1b", bufs=2) as p1b:
                axps = []
                for t in range(TT):
                    ap_t = psum.tile(
                        [P, 512], F32, tag="bank", name=f"axps{t}"
                    )
                    axps.append(ap_t)
                for k in range(KT):
                    lak = p1b.tile([P, 512], F32R, tag="lak", name="lak", bufs=4)
                    nc.sync.dma_start(lak[:], laT3[:, k, :])
                    for t in range(TT):
                        nc.tensor.matmul(
                            axps[t][:], xsb[:, k, ts(t, P)], lak[:],
                            start=(k == 0), stop=(k == KT - 1),
                        )
                axws = []
                for t in range(TT):
                    axw = p1b.tile(
                        [P, 512], F32R, tag=f"axw{t}", name=f"axw{t}", bufs=1
                    )
                    nc.vector.tensor_tensor(
                        axw[:].rearrange("p (e r) -> p e r", r=R),
                        axps[t][:].rearrange("p (e r) -> p e r", r=R),
                        wdense[t][:, :, None].to_broadcast([P, E, R]),
                        mybir.AluOpType.mult,
                    )
                    axws.append(axw)
                for t in range(TT):
                    tpq = psum.tile([P, 512], F32R, tag="bank", name="tpq")
                    for rr in range(RR):
                        nc.tensor.transpose(
                            tpq[:, ts(rr, P)], axws[t][:, ts(rr, P)],
                            identity[:],
                        )
                    nc.vector.tensor_copy(
                        axwT[:, :, ts(t, P)],
                        tpq[:].rearrange("p (rr q) -> p rr q", q=P),
                    )

            # ---- phase 2: base + delta per output tile ----
            KPC = KT // KC  # k-slices per base-weight chunk
            with (
                tc.tile_pool(name="p2bw", bufs=6) as p2bw,
                tc.tile_pool(name="p2lb", bufs=3) as p2lb,
                tc.tile_pool(name="p2o", bufs=4) as p2o,
            ):

                def load_lb(o):
                    lb = p2lb.tile([P, RR, 512], F32R, tag="lb", name="lb")
                    nc.sync.dma_start(lb[:], lbT3[:, :, ds(o * 512, 512)])
                    return lb

                def load_bwc(o, kc):
                    bwc = p2bw.tile([P, KPC, 512], F32R, tag="bwc", name="bwc")
                    nc.sync.dma_start(
                        bwc[:],
                        bwT2[
                            ds(kc * KPC * P, KPC * P), ds(o * 512, 512)
                        ].rearrange("(kk p) o -> p kk o", p=P),
                    )
                    return bwc

                lb_next = load_lb(0)
                bw_pre = {0: load_bwc(0, 0), 1: load_bwc(0, 1)}
                for o in range(OT):
                    lb = lb_next
                    ps2 = {}
                    for kc in range(KC):
                        bwc = bw_pre.pop(kc, None)
                        if bwc is None:
                            bwc = load_bwc(o, kc)
                        for t in range(TT):
                            if kc == 0:
                                ps2[t] = psum.tile(
                                    [P, 512], F32, tag="bank",
                                    name=f"ps2_{o}_{t}",
                                )
                            for k in range(KPC):
                                nc.tensor.matmul(
                                    ps2[t][:],
                                    xsb[:, kc * KPC + k, ts(t, P)],
                                    bwc[:, k, :],
                                    start=(kc == 0 and k == 0),
                                    stop=False,
                                )
                    # prefetch next o ahead of this o's output burst
                    if o + 1 < OT:
                        lb_next = load_lb(o + 1)
                        bw_pre = {
                            0: load_bwc(o + 1, 0),
                            1: load_bwc(o + 1, 1),
                        }
                    for t in range(TT):
                        for rr in range(RR):
                            nc.tensor.matmul(
                                ps2[t][:],
                                axwT[:, rr, ts(t, P)],
                                lb[:, rr, :],
                                start=False,
                                stop=(rr == RR - 1),
                            )
                        osb = p2o.tile([P, 512], F32, tag="osb", name="osb")
                        nc.vector.tensor_copy(osb[:], ps2[t][:])
                        nc.sync.dma_start(
                            out2[ts(t, P), ds(o * 512, 512)], osb[:]
                        )

    nc.compile()
    return nc


def _get_nc():
    if "nc" not in _CACHE:
        _CACHE["nc"] = _build()
    return _CACHE["nc"]


def kernel(x, base_w, gate_w, lora_A, lora_B):
    nc = _get_nc()

    x2 = np.ascontiguousarray(np.asarray(x, dtype=np.float32).reshape(B * S, DIN))
    bwT = np.ascontiguousarray(np.asarray(base_w, dtype=np.float32).T)
    gwT = np.ascontiguousarray(np.asarray(gate_w, dtype=np.float32).T)
    laT = np.ascontiguousarray(np.asarray(lora_A, dtype=np.float32).T)
    lbT = np.ascontiguousarray(
        np.asarray(lora_B, dtype=np.float32).T * np.float32(SCALING)
    )
    iden = np.eye(P, dtype=np.float32)

    in_maps = []
    for c in range(NCORES):
        xT_c = np.ascontiguousarray(x2[c * T : (c + 1) * T].T)
        in_maps.append(
            {
                "xT": xT_c,
                "bwT": bwT,
                "gwT": gwT,
                "laT": laT,
                "lbT": lbT,
                "iden": iden,
            }
        )

    res = bass_utils.run_bass_kernel_spmd(nc, in_maps, core_ids=list(range(NCORES)))
    parts = [res.results[c]["out"] for c in range(NCORES)]
    return np.concatenate(parts, axis=0).reshape(B, S, DOUT).astype(np.float32)



# revision 7
# speedup vs baseline: 1.4342x; 1.4342x over previous
"""MoE-LoRA linear kernel for Trainium2 (8 NeuronCores, data-parallel over tokens).

Computes, for x:[B,S,Din], base_w:[Dout,Din], gate_w:[E,Din],
lora_A:[E*R,Din], lora_B:[Dout,E*R]:

    base   = x @ base_w.T
    logits = x @ gate_w.T ; top-2 renormalized softmax -> dense w:[*,E]
    ax     = x @ lora_A.T                 (per-expert rank-R blocks)
    delta  = (ax * w_expanded) @ lora_B.T * SCALING
    out    = base + delta

Sharding: tokens (B*S=8192) split across 8 cores, 1024 tokens each.
Weights replicated. No collectives.

Precision scheme (validated ~7e-3 rel-err vs fp64, gate is 2e-2):
  - base: 3-term error-compensated fp8e4 DoubleRow matmuls
        x8@w8 + dx8@w8 + x8@dw8
    where x8=e4m3(x), dx8=e4m3(x-x8), w8=e4m3(32*base_w.T),
    dw8=e4m3(32*base_w.T - w8). PSUM holds 32*out; final copy scales 1/32.
  - gating: fp16 matmuls (fp32 PSUM accumulate), exact top-2 via DVE Max8 +
    sigmoid identity; dense gate weights carry a folded 1/32.
  - ax: naive fp8 DoubleRow vs lA8=e4m3(32*lora_A.T); result parked to SBUF
    fp16, multiplied by gate/32 -> axw fp8 (true scale), PE-transposed.
  - delta: fp8 DoubleRow vs lB8=e4m3(64*lora_B.T) accumulating into the same
    PSUM group as base (64 = SCALING*32).

DoubleRow packs 2 k-tiles per matmul (K=256 per instruction), both operands
laid out [128, 2, F].
"""
import sys

if "/opt/trn_rl_repo" not in sys.path:
    sys.path.insert(0, "/opt/trn_rl_repo")

import numpy as np
import ml_dtypes

import concourse.bacc as bacc
import concourse.mybir as mybir
import concourse.tile as tile
from concourse import bass_utils
from concourse.bass import ds, ts

B, S, DIN, DOUT = 4, 2048, 4096, 4096
E, R = 32, 16
SCALING = 2.0
SW = 32.0          # weight prescale (subnormal avoidance)
NCORES = 8
T = (B * S) // NCORES  # 1024 tokens per core
P = 128
TT = T // P            # 8 token tiles
KT = DIN // P          # 32 contraction tiles of 128
KC = KT // 2           # 16 DoubleRow chunks of 256
OT = DOUT // 512       # 8 output column tiles
RR = (E * R) // P      # 4 rank tiles
RC = RR // 2           # 2 rank DoubleRow chunks
KG = 4                 # k-group DMA split (8 k-tiles each)
KGT = KT // KG
F32 = mybir.dt.float32
F16 = mybir.dt.float16
F8 = mybir.dt.float8e4
DR = mybir.MatmulPerfMode.DoubleRow
E4NP = ml_dtypes.float8_e4m3

_CACHE = {}


def _build():
    nc = bacc.Bacc("TRN2", target_bir_lowering=False, debug=False)
    x8T = nc.dram_tensor("x8T", [DIN, T], F8, kind="ExternalInput")
    dx8T = nc.dram_tensor("dx8T", [DIN, T], F8, kind="ExternalInput")
    x16T = nc.dram_tensor("x16T", [DIN, T], F16, kind="ExternalInput")
    gw16T = nc.dram_tensor("gw16T", [DIN, E], F16, kind="ExternalInput")
    w8T = nc.dram_tensor("w8T", [DIN, DOUT], F8, kind="ExternalInput")
    dw8T = nc.dram_tensor("dw8T", [DIN, DOUT], F8, kind="ExternalInput")
    lA8T = nc.dram_tensor("lA8T", [DIN, E * R], F8, kind="ExternalInput")
    lB8T = nc.dram_tensor("lB8T", [E * R, DOUT], F8, kind="ExternalInput")
    iden = nc.dram_tensor("iden", [P, P], F16, kind="ExternalInput")
    out = nc.dram_tensor("out", [T, DOUT], F32, kind="ExternalOutput")

    x8T3 = x8T.ap().rearrange("(k p) t -> p k t", p=P)
    dx8T3 = dx8T.ap().rearrange("(k p) t -> p k t", p=P)
    x16T3 = x16T.ap().rearrange("(k p) t -> p k t", p=P)
    gwT3 = gw16T.ap().rearrange("(k p) e -> p k e", p=P)
    laT3 = lA8T.ap().rearrange("(k p) r -> p k r", p=P)
    lbT3 = lB8T.ap().rearrange("(rr p) o -> p rr o", p=P)
    w8T4 = w8T.ap().rearrange("(kc j p) o -> p kc j o", p=P, j=2)
    dw8T4 = dw8T.ap().rearrange("(kc j p) o -> p kc j o", p=P, j=2)
    out2 = out.ap()

    with tile.TileContext(nc, pool_alloc_mode="queue") as tc:
        with (
            tc.tile_pool(name="base", bufs=1) as bp,
            tc.tile_pool(name="psum", bufs=8, space="PSUM") as psum,
        ):
            identity = bp.tile([P, P], F16, tag="iden")
            x8sb = bp.tile([P, KT, T], F8, tag="x8sb")
            dx8sb = bp.tile([P, KT, T], F8, tag="dx8sb")
            lA8sb = bp.tile([P, KT, E * R], F8, tag="lA8sb")
            gw16 = bp.tile([P, KT, E], F16, tag="gw16")
            axwT8 = bp.tile([P, RR, T], F8, tag="axwT8")
            wdense = []
            axsb = []
            for t in range(TT):
                wdense.append(bp.tile([P, E], F32, tag=f"wd{t}", name=f"wd{t}"))
                axsb.append(
                    bp.tile([P, E * R], F16, tag=f"axsb{t}", name=f"axsb{t}")
                )

            # ---- DMA schedule (order matters: the SP queue drains in order
            # and phase 1 is DMA-bound) ----
            nc.sync.dma_start(identity[:], iden.ap())
            nc.sync.dma_start(gw16[:], gwT3)
            with tc.tile_pool(name="p1a", bufs=2) as p1a:
                x16g = []
                for g in range(KG):
                    xg = p1a.tile(
                        [P, KGT, T], F16, tag=f"x16g{g}", name=f"x16g{g}",
                        bufs=1,
                    )
                    x16g.append(xg)
                # first gating chunk, then x8 + lA8 (unblocks ax), then the
                # rest of gating, then dx8 (only needed by phase 2)
                nc.sync.dma_start(x16g[0][:], x16T3[:, :KGT, :])
                for g in range(KG):
                    nc.sync.dma_start(
                        x8sb[:, ds(g * KGT, KGT), :], x8T3[:, ds(g * KGT, KGT), :]
                    )
                nc.sync.dma_start(lA8sb[:], laT3)
                for g in range(1, KG):
                    nc.sync.dma_start(
                        x16g[g][:], x16T3[:, ds(g * KGT, KGT), :]
                    )
                for g in range(KG):
                    nc.sync.dma_start(
                        dx8sb[:, ds(g * KGT, KGT), :],
                        dx8T3[:, ds(g * KGT, KGT), :],
                    )

                # ---- phase 1a-part1: gating partial sums for g=0; then ax
                # (PE work while the rest of x16 streams in) ----
                lsb = []
                for t in range(TT):
                    ls = p1a.tile([P, E], F32, tag=f"lsb{t}", name=f"lsb{t}", bufs=1)
                    lsb.append(ls)

                def gating_block(g):
                    for t in range(TT):
                        pl = p1a_psum_tile()
                        for k in range(KGT):
                            nc.tensor.matmul(
                                pl[:],
                                x16g[g][:, k, ts(t, P)],
                                gw16[:, g * KGT + k, :],
                                start=(k == 0),
                                stop=(k == KGT - 1),
                            )
                        if g == 0:
                            nc.vector.tensor_copy(lsb[t][:], pl[:])
                        else:
                            nc.vector.tensor_add(lsb[t][:], lsb[t][:], pl[:])

                def p1a_psum_tile():
                    return psum.tile([P, E], F32, tag="bank", name="pl")

                gating_block(0)

                # ax: fp8 DoubleRow, full rank width, one psum bank per t,
                # parked to SBUF fp16 immediately (gate weights arrive later)
                for t in range(TT):
                    axps = psum.tile([P, E * R], F32, tag="bank", name="axps")
                    for kc in range(KC):
                        nc.tensor.matmul(
                            axps[:],
                            x8sb[:, ds(2 * kc, 2), ts(t, P)],
                            lA8sb[:, ds(2 * kc, 2), :],
                            start=(kc == 0),
                            stop=(kc == KC - 1),
                            perf_mode=DR,
                        )
                    nc.vector.tensor_copy(axsb[t][:], axps[:])

                for g in range(1, KG):
                    gating_block(g)

                # ---- gating math: top-2 renormalized softmax with folded 1/32
                for t in range(TT):
                    m8 = p1a.tile([P, 8], F32, tag="m8", name="m8")
                    nc.vector.max(out=m8[:], in_=lsb[t][:])
                    d21 = p1a.tile([P, 1], F32, tag="d21", name="d21")
                    nc.vector.tensor_sub(d21[:], m8[:, 1:2], m8[:, 0:1])
                    e2 = p1a.tile([P, 1], F32, tag="e2", name="e2")
                    nc.scalar.activation(
                        e2[:], d21[:], mybir.ActivationFunctionType.Exp
                    )
                    den = p1a.tile([P, 1], F32, tag="den", name="den")
                    nc.vector.tensor_scalar(
                        den[:], e2[:], SW, SW,
                        mybir.AluOpType.mult, mybir.AluOpType.add,
                    )
                    w1 = p1a.tile([P, 1], F32, tag="w1", name="w1")
                    nc.vector.reciprocal(w1[:], den[:])
                    w2 = p1a.tile([P, 1], F32, tag="w2", name="w2")
                    nc.vector.tensor_mul(w2[:], e2[:], w1[:])
                    eq1 = p1a.tile([P, E], F32, tag="eq1", name="eq1")
                    nc.vector.tensor_tensor(
                        eq1[:], lsb[t][:], m8[:, 0:1].to_broadcast([P, E]),
                        mybir.AluOpType.is_equal,
                    )
                    eq2 = p1a.tile([P, E], F32, tag="eq2", name="eq2")
                    nc.vector.tensor_tensor(
                        eq2[:], lsb[t][:], m8[:, 1:2].to_broadcast([P, E]),
                        mybir.AluOpType.is_equal,
                    )
                    nc.vector.tensor_tensor(
                        eq1[:], eq1[:], w1[:].to_broadcast([P, E]),
                        mybir.AluOpType.mult,
                    )
                    nc.vector.tensor_tensor(
                        eq2[:], eq2[:], w2[:].to_broadcast([P, E]),
                        mybir.AluOpType.mult,
                    )
                    nc.vector.tensor_add(wdense[t][:], eq1[:], eq2[:])

            # ---- phase 1b: axw = ax * gate/32 -> fp8, PE transpose ----
            with tc.tile_pool(name="p1b", bufs=2) as p1b:
                for t in range(TT):
                    axw = p1b.tile([P, E * R], F16, tag="axw", name="axw", bufs=3)
                    nc.vector.tensor_tensor(
                        axw[:].rearrange("p (e r) -> p e r", r=R),
                        axsb[t][:].rearrange("p (e r) -> p e r", r=R),
                        wdense[t][:, :, None].to_broadcast([P, E, R]),
                        mybir.AluOpType.mult,
                    )
                    tpq = psum.tile([P, E * R], F16, tag="bank", name="tpq")
                    for rr in range(RR):
                        nc.tensor.transpose(
                            tpq[:, ts(rr, P)], axw[:, ts(rr, P)], identity[:]
                        )
                    nc.vector.tensor_copy(
                        axwT8[:, :, ts(t, P)],
                        tpq[:].rearrange("p (rr q) -> p rr q", q=P),
                    )

            # ---- phase 2: base (3-term fp8 DR) + delta per output tile ----
            with (
                tc.tile_pool(name="p2w", bufs=6) as p2w,
                tc.tile_pool(name="p2lb", bufs=3) as p2lb,
                tc.tile_pool(name="p2o", bufs=4) as p2o,
            ):

                def load_lb(o):
                    lb = p2lb.tile([P, RR, 512], F8, tag="lb", name="lb")
                    nc.sync.dma_start(lb[:], lbT3[:, :, ds(o * 512, 512)])
                    return lb

                def load_wc(o, kc):
                    wc = p2w.tile([P, 2, 512], F8, tag="wc", name="wc")
                    nc.sync.dma_start(wc[:], w8T4[:, kc, :, ds(o * 512, 512)])
                    dwc = p2w.tile([P, 2, 512], F8, tag="dwc", name="dwc")
                    nc.sync.dma_start(dwc[:], dw8T4[:, kc, :, ds(o * 512, 512)])
                    return wc, dwc

                lb_next = load_lb(0)
                wc_pre = {0: load_wc(0, 0), 1: load_wc(0, 1)}
                for o in range(OT):
                    lb = lb_next
                    ps2 = {}
                    for kc in range(KC):
                        pair = wc_pre.pop(kc, None)
                        if pair is None:
                            pair = load_wc(o, kc)
                        wc, dwc = pair
                        for t in range(TT):
                            if kc == 0:
                                ps2[t] = psum.tile(
                                    [P, 512], F32, tag="bank",
                                    name=f"ps2_{o}_{t}",
                                )
                            xs = x8sb[:, ds(2 * kc, 2), ts(t, P)]
                            dxs = dx8sb[:, ds(2 * kc, 2), ts(t, P)]
                            nc.tensor.matmul(
                                ps2[t][:], xs, wc[:],
                                start=(kc == 0), stop=False, perf_mode=DR,
                            )
                            nc.tensor.matmul(
                                ps2[t][:], dxs, wc[:],
                                start=False, stop=False, perf_mode=DR,
                            )
                            nc.tensor.matmul(
                                ps2[t][:], xs, dwc[:],
                                start=False, stop=False, perf_mode=DR,
                            )
                    # prefetch next o ahead of this o's output burst
                    if o + 1 < OT:
                        lb_next = load_lb(o + 1)
                        wc_pre = {
                            0: load_wc(o + 1, 0),
                            1: load_wc(o + 1, 1),
                        }
                    for t in range(TT):
                        for c in range(RC):
                            nc.tensor.matmul(
                                ps2[t][:],
                                axwT8[:, ds(2 * c, 2), ts(t, P)],
                                lb[:, ds(2 * c, 2), :],
                                start=False, stop=(c == RC - 1),
                                perf_mode=DR,
                            )
                        osb = p2o.tile([P, 512], F32, tag="osb", name="osb")
                        nc.vector.tensor_scalar_mul(osb[:], ps2[t][:], 1.0 / SW)
                        nc.sync.dma_start(
                            out2[ts(t, P), ds(o * 512, 512)], osb[:]
                        )

    nc.compile()
    return nc


def _get_nc():
    if "nc" not in _CACHE:
        _CACHE["nc"] = _build()
    return _CACHE["nc"]


def kernel(x, base_w, gate_w, lora_A, lora_B):
    nc = _get_nc()

    f32 = np.float32
    x2 = np.asarray(x, dtype=f32).reshape(B * S, DIN)
    wT = np.ascontiguousarray(np.asarray(base_w, dtype=f32).T) * f32(SW)
    w8 = wT.astype(E4NP)
    dw8 = (wT - w8.astype(f32)).astype(E4NP)
    gw16 = np.ascontiguousarray(np.asarray(gate_w, dtype=f32).T).astype(
        np.float16
    )
    lA8 = (
        np.ascontiguousarray(np.asarray(lora_A, dtype=f32).T) * f32(SW)
    ).astype(E4NP)
    lB8 = (
        np.ascontiguousarray(np.asarray(lora_B, dtype=f32).T)
        * f32(SCALING * SW)
    ).astype(E4NP)
    iden = np.eye(P, dtype=np.float16)

    in_maps = []
    for c in range(NCORES):
        xT_c = np.ascontiguousarray(x2[c * T : (c + 1) * T].T)
        x8_c = xT_c.astype(E4NP)
        dx8_c = (xT_c - x8_c.astype(f32)).astype(E4NP)
        x16_c = xT_c.astype(np.float16)
        in_maps.append(
            {
                "x8T": x8_c,
                "dx8T": dx8_c,
                "x16T": x16_c,
                "gw16T": gw16,
                "w8T": w8,
                "dw8T": dw8,
                "lA8T": lA8,
                "lB8T": lB8,
                "iden": iden,
            }
        )

    res = bass_utils.run_bass_kernel_spmd(nc, in_maps, core_ids=list(range(NCORES)))
    parts = [res.results[c]["out"] for c in range(NCORES)]
    return np.concatenate(parts, axis=0).reshape(B, S, DOUT).astype(np.float32)


# revision 8
# speedup vs baseline: 1.5292x; 1.0663x over previous
"""MoE-LoRA linear kernel for Trainium2 (8 NeuronCores, data-parallel over tokens).

Computes, for x:[B,S,Din], base_w:[Dout,Din], gate_w:[E,Din],
lora_A:[E*R,Din], lora_B:[Dout,E*R]:

    base   = x @ base_w.T
    logits = x @ gate_w.T ; top-2 renormalized softmax -> dense w:[*,E]
    ax     = x @ lora_A.T                 (per-expert rank-R blocks)
    delta  = (ax * w_expanded) @ lora_B.T * SCALING
    out    = base + delta

Sharding: tokens (B*S=8192) split across 8 cores, 1024 tokens each.
Weights replicated. No collectives.

Precision scheme (validated ~8e-3 rel-err vs fp64, gate is 2e-2):
  - base & gating: 3-term error-compensated fp8e4 DoubleRow matmuls
        x8@w8 + dx8@w8 + x8@dw8
    where x8=e4m3(x), dx8=e4m3(x-x8), w8=e4m3(32*W.T), dw8=e4m3(32*W.T-w8).
    PSUM holds 32x the true value; the output copy scales by 1/32, gating
    folds 1/32 into the dense gate weights and exp(d/32) into the ACT scale.
  - ax: naive fp8 DoubleRow vs lA8=e4m3(32*lora_A.T); parked to SBUF fp16,
    multiplied by gate/32 -> axw fp8 (true scale), PE-transposed (fp16).
  - delta: fp8 DoubleRow vs lB8=e4m3(64*lora_B.T) accumulated into the same
    PSUM group as base (64 = SCALING*32).

Schedule: phase-1 work (gating, ax, transposes) is interleaved with the
(o=0, t=0..5) base accumulation so the PE stays busy while x8/dx8/lA8
stream in. PSUM is split into a 6-bank pool (long-lived base groups) and a
2-bank pool (transients + the t=6..7 groups, whose o=0 pass re-streams the
weight chunks).
"""
import sys

if "/opt/trn_rl_repo" not in sys.path:
    sys.path.insert(0, "/opt/trn_rl_repo")

import numpy as np
import ml_dtypes

import concourse.bacc as bacc
import concourse.mybir as mybir
import concourse.tile as tile
from concourse import bass_utils
from concourse.bass import ds, ts

B, S, DIN, DOUT = 4, 2048, 4096, 4096
E, R = 32, 16
SCALING = 2.0
SW = 32.0          # weight prescale (subnormal avoidance)
NCORES = 8
T = (B * S) // NCORES  # 1024 tokens per core
P = 128
TT = T // P            # 8 token tiles
TA = 6                 # token tiles in the 6-bank psum pool
KT = DIN // P          # 32 contraction tiles of 128
KC = KT // 2           # 16 DoubleRow chunks of 256
OT = DOUT // 512       # 8 output column tiles
RR = (E * R) // P      # 4 rank tiles
RC = RR // 2           # 2 rank DoubleRow chunks
G = 4                  # prologue blocks / dx8 DMA groups (8 k-tiles each)
KGT = KT // G
F32 = mybir.dt.float32
F16 = mybir.dt.float16
F8 = mybir.dt.float8e4
DR = mybir.MatmulPerfMode.DoubleRow
COPY = mybir.ActivationFunctionType.Copy
EXP = mybir.ActivationFunctionType.Exp
E4NP = ml_dtypes.float8_e4m3

_CACHE = {}


def _build():
    nc = bacc.Bacc("TRN2", target_bir_lowering=False, debug=False)
    x8T = nc.dram_tensor("x8T", [DIN, T], F8, kind="ExternalInput")
    dx8T = nc.dram_tensor("dx8T", [DIN, T], F8, kind="ExternalInput")
    gw8T = nc.dram_tensor("gw8T", [DIN, E], F8, kind="ExternalInput")
    dgw8T = nc.dram_tensor("dgw8T", [DIN, E], F8, kind="ExternalInput")
    w8T = nc.dram_tensor("w8T", [DIN, DOUT], F8, kind="ExternalInput")
    dw8T = nc.dram_tensor("dw8T", [DIN, DOUT], F8, kind="ExternalInput")
    lA8T = nc.dram_tensor("lA8T", [DIN, E * R], F8, kind="ExternalInput")
    lB8T = nc.dram_tensor("lB8T", [E * R, DOUT], F8, kind="ExternalInput")
    iden = nc.dram_tensor("iden", [P, P], F16, kind="ExternalInput")
    out = nc.dram_tensor("out", [T, DOUT], F32, kind="ExternalOutput")

    x8T3 = x8T.ap().rearrange("(k p) t -> p k t", p=P)
    dx8T3 = dx8T.ap().rearrange("(k p) t -> p k t", p=P)
    gwT3 = gw8T.ap().rearrange("(k p) e -> p k e", p=P)
    dgwT3 = dgw8T.ap().rearrange("(k p) e -> p k e", p=P)
    laT3 = lA8T.ap().rearrange("(k p) r -> p k r", p=P)
    lbT3 = lB8T.ap().rearrange("(rr p) o -> p rr o", p=P)
    w8T4 = w8T.ap().rearrange("(kc j p) o -> p kc j o", p=P, j=2)
    dw8T4 = dw8T.ap().rearrange("(kc j p) o -> p kc j o", p=P, j=2)
    out2 = out.ap()

    with tile.TileContext(nc, pool_alloc_mode="queue") as tc:
        with (
            tc.tile_pool(name="base", bufs=1) as bp,
            tc.tile_pool(name="psumA", bufs=6, space="PSUM") as psumA,
            tc.tile_pool(name="psumB", bufs=2, space="PSUM") as psumB,
            tc.tile_pool(name="p2w", bufs=16) as p2w,
            tc.tile_pool(name="p2lb", bufs=3) as p2lb,
            tc.tile_pool(name="p2o", bufs=6) as p2o,
            tc.tile_pool(name="p1x", bufs=2) as p1x,
        ):
            identity = bp.tile([P, P], F16, tag="iden")
            x8sb = bp.tile([P, KT, T], F8, tag="x8sb")
            dx8sb = bp.tile([P, KT, T], F8, tag="dx8sb")
            lA8sb = bp.tile([P, KT, E * R], F8, tag="lA8sb")
            gw8sb = bp.tile([P, KT, E], F8, tag="gw8sb")
            dgw8sb = bp.tile([P, KT, E], F8, tag="dgw8sb")
            axwT8 = bp.tile([P, RR, T], F8, tag="axwT8")
            wdense, axsb, lsb = [], [], []
            for t in range(TT):
                wdense.append(bp.tile([P, E], F32, tag=f"wd{t}", name=f"wd{t}"))
                axsb.append(
                    bp.tile([P, E * R], F16, tag=f"axsb{t}", name=f"axsb{t}")
                )
                lsb.append(bp.tile([P, E], F32, tag=f"lsb{t}", name=f"lsb{t}"))

            def load_wc(o, kc):
                wc = p2w.tile([P, 2, 512], F8, tag="wc", name="wc")
                nc.sync.dma_start(wc[:], w8T4[:, kc, :, ds(o * 512, 512)])
                dwc = p2w.tile([P, 2, 512], F8, tag="dwc", name="dwc")
                nc.sync.dma_start(dwc[:], dw8T4[:, kc, :, ds(o * 512, 512)])
                return wc, dwc

            def load_lb(o):
                lb = p2lb.tile([P, RR, 512], F8, tag="lb", name="lb")
                nc.sync.dma_start(lb[:], lbT3[:, :, ds(o * 512, 512)])
                return lb

            # ---- DMA ladder: paced so the interleaved prologue never
            # starves. x8/wc(o0)/dx8 per k-group, then lA8/lb. ----
            wc0 = {}
            for g in range(G):
                nc.sync.dma_start(
                    x8sb[:, ds(g * KGT, KGT), :], x8T3[:, ds(g * KGT, KGT), :]
                )
                for kc in range(4 * g, 4 * g + 4):
                    wc0[kc] = load_wc(0, kc)
                nc.sync.dma_start(
                    dx8sb[:, ds(g * KGT, KGT), :], dx8T3[:, ds(g * KGT, KGT), :]
                )
                if g == 0:
                    nc.sync.dma_start(identity[:], iden.ap())
                    nc.sync.dma_start(gw8sb[:], gwT3)
                    nc.sync.dma_start(dgw8sb[:], dgwT3)
                if g == 1:
                    nc.sync.dma_start(lA8sb[:], laT3)
            lb0 = load_lb(0)

            # ---- prologue: (o0, t0..5) base + gating, interleaved ----
            ps2 = {}
            for g in range(G):
                for kc in range(4 * g, 4 * g + 4):
                    wc, dwc = wc0.pop(kc)
                    for t in range(TA):
                        if kc == 0:
                            ps2[t] = psumA.tile(
                                [P, 512], F32, tag="bankA", name=f"psA{t}"
                            )
                        xs = x8sb[:, ds(2 * kc, 2), ts(t, P)]
                        nc.tensor.matmul(
                            ps2[t][:], xs, wc[:],
                            start=(kc == 0), stop=False, perf_mode=DR,
                        )
                        nc.tensor.matmul(
                            ps2[t][:], xs, dwc[:],
                            start=False, stop=False, perf_mode=DR,
                        )
                    wc0[kc] = (wc, dwc)
                # dx8 terms second (dx8 group lands after the wc chunks)
                for kc in range(4 * g, 4 * g + 4):
                    wc, _ = wc0.pop(kc)
                    for t in range(TA):
                        nc.tensor.matmul(
                            ps2[t][:],
                            dx8sb[:, ds(2 * kc, 2), ts(t, P)],
                            wc[:],
                            start=False, stop=False, perf_mode=DR,
                        )
                # gating chunks 4g..4g+3 (3-term DR, partial sums via DVE)
                for t in range(TT):
                    pl = psumB.tile([P, 512], F32, tag="bankB", name="pl")
                    pe = pl[:, :E]
                    for i, c in enumerate(range(4 * g, 4 * g + 4)):
                        xs = x8sb[:, ds(2 * c, 2), ts(t, P)]
                        dxs = dx8sb[:, ds(2 * c, 2), ts(t, P)]
                        gs = gw8sb[:, ds(2 * c, 2), :]
                        dgs = dgw8sb[:, ds(2 * c, 2), :]
                        nc.tensor.matmul(
                            pe, xs, gs, start=(i == 0), stop=False,
                            perf_mode=DR,
                        )
                        nc.tensor.matmul(
                            pe, dxs, gs, start=False, stop=False, perf_mode=DR
                        )
                        nc.tensor.matmul(
                            pe, xs, dgs, start=False, stop=(i == 3),
                            perf_mode=DR,
                        )
                    if g == 0:
                        nc.vector.tensor_copy(lsb[t][:], pe)
                    else:
                        nc.vector.tensor_add(lsb[t][:], lsb[t][:], pe)

            # ---- gating math: top-2 renormalized softmax, 1/32 folded ----
            for t in range(TT):
                m8 = p1x.tile([P, 8], F32, tag="m8", name="m8")
                nc.vector.max(out=m8[:], in_=lsb[t][:])
                d21 = p1x.tile([P, 1], F32, tag="d21", name="d21")
                nc.vector.tensor_sub(d21[:], m8[:, 1:2], m8[:, 0:1])
                e2 = p1x.tile([P, 1], F32, tag="e2", name="e2")
                nc.scalar.activation(e2[:], d21[:], EXP, scale=1.0 / SW)
                den = p1x.tile([P, 1], F32, tag="den", name="den")
                nc.vector.tensor_scalar(
                    den[:], e2[:], SW, SW,
                    mybir.AluOpType.mult, mybir.AluOpType.add,
                )
                w1 = p1x.tile([P, 1], F32, tag="w1", name="w1")
                nc.vector.reciprocal(w1[:], den[:])
                w2 = p1x.tile([P, 1], F32, tag="w2", name="w2")
                nc.vector.tensor_mul(w2[:], e2[:], w1[:])
                eq1 = p1x.tile([P, E], F32, tag="eq1", name="eq1")
                nc.vector.tensor_tensor(
                    eq1[:], lsb[t][:], m8[:, 0:1].to_broadcast([P, E]),
                    mybir.AluOpType.is_equal,
                )
                eq2 = p1x.tile([P, E], F32, tag="eq2", name="eq2")
                nc.vector.tensor_tensor(
                    eq2[:], lsb[t][:], m8[:, 1:2].to_broadcast([P, E]),
                    mybir.AluOpType.is_equal,
                )
                nc.vector.tensor_tensor(
                    eq1[:], eq1[:], w1[:].to_broadcast([P, E]),
                    mybir.AluOpType.mult,
                )
                nc.vector.tensor_tensor(
                    eq2[:], eq2[:], w2[:].to_broadcast([P, E]),
                    mybir.AluOpType.mult,
                )
                nc.vector.tensor_add(wdense[t][:], eq1[:], eq2[:])

            # ---- ax (fp8 DR), parked to SBUF fp16 ----
            for t in range(TT):
                axps = psumB.tile([P, E * R], F32, tag="bankB", name="axps")
                for kc in range(KC):
                    nc.tensor.matmul(
                        axps[:],
                        x8sb[:, ds(2 * kc, 2), ts(t, P)],
                        lA8sb[:, ds(2 * kc, 2), :],
                        start=(kc == 0), stop=(kc == KC - 1), perf_mode=DR,
                    )
                nc.vector.tensor_copy(axsb[t][:], axps[:])

            # ---- axw = ax * gate/32 -> fp8, PE transpose (fp16) ----
            for t in range(TT):
                axw = p1x.tile([P, E * R], F16, tag="axw", name="axw", bufs=3)
                nc.vector.tensor_tensor(
                    axw[:].rearrange("p (e r) -> p e r", r=R),
                    axsb[t][:].rearrange("p (e r) -> p e r", r=R),
                    wdense[t][:, :, None].to_broadcast([P, E, R]),
                    mybir.AluOpType.mult,
                )
                tpq = psumB.tile([P, 2 * E * R], F16, tag="bankB", name="tpq")
                for rr in range(RR):
                    nc.tensor.transpose(
                        tpq[:, ts(rr, P)], axw[:, ts(rr, P)], identity[:]
                    )
                nc.vector.tensor_copy(
                    axwT8[:, :, ts(t, P)],
                    tpq[:, : E * R].rearrange("p (rr q) -> p rr q", q=P),
                )

            def delta_and_out(o, t, pst, lb):
                for c in range(RC):
                    nc.tensor.matmul(
                        pst[:],
                        axwT8[:, ds(2 * c, 2), ts(t, P)],
                        lb[:, ds(2 * c, 2), :],
                        start=False, stop=(c == RC - 1), perf_mode=DR,
                    )
                osb = p2o.tile([P, 512], F32, tag="osb", name="osb")
                nc.scalar.activation(osb[:], pst[:], COPY, scale=1.0 / SW)
                nc.sync.dma_start(out2[ts(t, P), ds(o * 512, 512)], osb[:])

            # ---- o0 tail: delta for t0..5; re-stream weights for t6..7 ----
            for t in range(TA):
                delta_and_out(0, t, ps2[t], lb0)
            ps2b = {}
            for kc in range(KC):
                wc, dwc = wc0.pop(kc) if kc in wc0 else load_wc(0, kc)
                for t in range(TA, TT):
                    if kc == 0:
                        ps2b[t] = psumB.tile(
                            [P, 512], F32, tag="bankB", name=f"psB{t}"
                        )
                    xs = x8sb[:, ds(2 * kc, 2), ts(t, P)]
                    dxs = dx8sb[:, ds(2 * kc, 2), ts(t, P)]
                    nc.tensor.matmul(
                        ps2b[t][:], xs, wc[:],
                        start=(kc == 0), stop=False, perf_mode=DR,
                    )
                    nc.tensor.matmul(
                        ps2b[t][:], dxs, wc[:],
                        start=False, stop=False, perf_mode=DR,
                    )
                    nc.tensor.matmul(
                        ps2b[t][:], xs, dwc[:],
                        start=False, stop=False, perf_mode=DR,
                    )
            lb_next = load_lb(1)
            wc_pre = {0: load_wc(1, 0), 1: load_wc(1, 1)}
            for t in range(TA, TT):
                delta_and_out(0, t, ps2b[t], lb0)

            # ---- o1..7: full 8-bank passes ----
            for o in range(1, OT):
                lb = lb_next
                ps2 = {}
                for kc in range(KC):
                    pair = wc_pre.pop(kc, None)
                    if pair is None:
                        pair = load_wc(o, kc)
                    wc, dwc = pair
                    for t in range(TT):
                        if kc == 0:
                            pool = psumA if t < TA else psumB
                            tg = "bankA" if t < TA else "bankB"
                            ps2[t] = pool.tile(
                                [P, 512], F32, tag=tg, name=f"ps2_{o}_{t}"
                            )
                        xs = x8sb[:, ds(2 * kc, 2), ts(t, P)]
                        dxs = dx8sb[:, ds(2 * kc, 2), ts(t, P)]
                        nc.tensor.matmul(
                            ps2[t][:], xs, wc[:],
                            start=(kc == 0), stop=False, perf_mode=DR,
                        )
                        nc.tensor.matmul(
                            ps2[t][:], dxs, wc[:],
                            start=False, stop=False, perf_mode=DR,
                        )
                        nc.tensor.matmul(
                            ps2[t][:], xs, dwc[:],
                            start=False, stop=False, perf_mode=DR,
                        )
                if o + 1 < OT:
                    lb_next = load_lb(o + 1)
                    wc_pre = {0: load_wc(o + 1, 0), 1: load_wc(o + 1, 1)}
                for t in range(TT):
                    delta_and_out(o, t, ps2[t], lb)

    nc.compile()
    return nc


def _get_nc():
    if "nc" not in _CACHE:
        _CACHE["nc"] = _build()
    return _CACHE["nc"]


def kernel(x, base_w, gate_w, lora_A, lora_B):
    nc = _get_nc()

    f32 = np.float32
    x2 = np.asarray(x, dtype=f32).reshape(B * S, DIN)
    wT = np.ascontiguousarray(np.asarray(base_w, dtype=f32).T) * f32(SW)
    w8 = wT.astype(E4NP)
    dw8 = (wT - w8.astype(f32)).astype(E4NP)
    gT = np.ascontiguousarray(np.asarray(gate_w, dtype=f32).T) * f32(SW)
    gw8 = gT.astype(E4NP)
    dgw8 = (gT - gw8.astype(f32)).astype(E4NP)
    lA8 = (
        np.ascontiguousarray(np.asarray(lora_A, dtype=f32).T) * f32(SW)
    ).astype(E4NP)
    lB8 = (
        np.ascontiguousarray(np.asarray(lora_B, dtype=f32).T)
        * f32(SCALING * SW)
    ).astype(E4NP)
    iden = np.eye(P, dtype=np.float16)

    in_maps = []
    for c in range(NCORES):
        xT_c = np.ascontiguousarray(x2[c * T : (c + 1) * T].T)
        x8_c = xT_c.astype(E4NP)
        dx8_c = (xT_c - x8_c.astype(f32)).astype(E4NP)
        in_maps.append(
            {
                "x8T": x8_c,
                "dx8T": dx8_c,
                "gw8T": gw8,
                "dgw8T": dgw8,
                "w8T": w8,
                "dw8T": dw8,
                "lA8T": lA8,
                "lB8T": lB8,
                "iden": iden,
            }
        )

    res = bass_utils.run_bass_kernel_spmd(nc, in_maps, core_ids=list(range(NCORES)))
    parts = [res.results[c]["out"] for c in range(NCORES)]
    return np.concatenate(parts, axis=0).reshape(B, S, DOUT).astype(np.float32)
